# revision 7
# baseline (speedup 1.0000x reference)
"""DeBERTa-MoE classifier on 8 TRN2 NeuronCores (Bass/Tile).

Strategy (single NEFF launch, SPMD over 8 cores):
  Phase 1 (data-parallel over batch): core c owns tokens [128c, 128c+128).
    - mean-pool its hidden shard over S via PE identity-accumulate matmuls
    - original classifier head tanh(cls@Wd+bd)@Wo+bo on its tokens
  AllGather the pooled activations m (bf16) so every core sees all tokens.
  Phase 2 (expert-parallel): core c owns experts {2c, 2c+1}. Only the
    top-4-routed tokens per expert are computed (capacity 384/expert):
    a host-built one-hot matrix gathers the selected rows via matmul,
    the expert MLP runs Linear->LN->GELU->Linear with the second Linear
    algebraically folded with the output projection (W2p = We2@Wp), and
    the transposed one-hot scatter-adds the weighted logits back.
  ReduceScatter sums the per-expert partial MoE logits; each core gets
  its own 128-token slice and runs the final LN classifier.
Host does only data staging (sharding, bf16 casts, transposes) and the
O(B*E) routing control-plane (top-k selection + softmax weights, f64).

Self-contained: hardcodes all shapes from the problem spec.
"""

import numpy as np
import ml_dtypes

import concourse.tile as tile
from concourse import bacc, mybir
from concourse.bass_utils import run_bass_kernel_spmd
from concourse.masks import make_identity

BF16 = ml_dtypes.bfloat16

B, S, H = 1024, 128, 1024
E, HID, TOPK, C = 16, 1024, 4, 3
EPS = 1e-5
NCORES = 8
TPC = B // NCORES          # tokens per core = 128
EPC = E // NCORES          # experts per core = 2
CAP = 384                  # per-expert token capacity (3 chunks of 128)
SLOTS = EPC * CAP          # 768 slots per core
JCH = SLOTS // 128         # 6 slot chunks
P = 128

_CACHE = {}


def _build():
    dt = mybir.dt
    nc = bacc.Bacc("TRN2", target_bir_lowering=False, debug=False,
                   num_devices=NCORES)

    # ---- I/O ----
    hsh = nc.dram_tensor("hsh", [TPC, S, H], dt.bfloat16, kind="ExternalInput")
    clsT = nc.dram_tensor("clsT", [H, TPC], dt.float32, kind="ExternalInput")
    Wd = nc.dram_tensor("Wd", [H, H], dt.float32, kind="ExternalInput")
    bd = nc.dram_tensor("bd", [1, H], dt.float32, kind="ExternalInput")
    Wo = nc.dram_tensor("Wo", [H, C], dt.float32, kind="ExternalInput")
    bo = nc.dram_tensor("bo", [1, C], dt.float32, kind="ExternalInput")
    We1 = nc.dram_tensor("We1", [EPC, H, HID], dt.bfloat16, kind="ExternalInput")
    be1 = nc.dram_tensor("be1", [1, EPC, HID], dt.float32, kind="ExternalInput")
    g1 = nc.dram_tensor("g1", [1, EPC, HID], dt.float32, kind="ExternalInput")
    beta1 = nc.dram_tensor("beta1", [1, EPC, HID], dt.float32, kind="ExternalInput")
    W2p = nc.dram_tensor("W2p", [EPC, HID, C], dt.float32, kind="ExternalInput")
    b2p = nc.dram_tensor("b2p", [1, EPC, C], dt.float32, kind="ExternalInput")
    Sg = nc.dram_tensor("Sg", [B, SLOTS], dt.bfloat16, kind="ExternalInput")
    SgT = nc.dram_tensor("SgT", [SLOTS, B], dt.float32, kind="ExternalInput")
    wsl = nc.dram_tensor("wsl", [P, JCH], dt.float32, kind="ExternalInput")
    Wf1 = nc.dram_tensor("Wf1", [2 * C, C], dt.float32, kind="ExternalInput")
    bf1 = nc.dram_tensor("bf1", [1, C], dt.float32, kind="ExternalInput")
    gf = nc.dram_tensor("gf", [1, C], dt.float32, kind="ExternalInput")
    betaf = nc.dram_tensor("betaf", [1, C], dt.float32, kind="ExternalInput")
    Wf2 = nc.dram_tensor("Wf2", [C, C], dt.float32, kind="ExternalInput")
    bf2 = nc.dram_tensor("bf2", [1, C], dt.float32, kind="ExternalInput")
    out_ext = nc.dram_tensor("out", [TPC, C], dt.float32, kind="ExternalOutput")

    rg = [list(range(NCORES))]

    with tile.TileContext(nc) as tc:
        with (
            tc.tile_pool(name="cst", bufs=1) as cst,
            tc.tile_pool(name="dram", bufs=1, space="DRAM") as dram,
        ):
            ident_b = cst.tile([P, P], dt.bfloat16)
            make_identity(nc, ident_b)
            ident_f = cst.tile([P, P], dt.float32)
            make_identity(nc, ident_f)
            eps_sb = cst.tile([P, 1], dt.float32)
            nc.vector.memset(eps_sb, EPS)

            ag_in = dram.tile([TPC, H], dt.bfloat16)
            ag_out = dram.tile([B, H], dt.bfloat16, addr_space="Shared")
            rs_in = dram.tile([B, C], dt.float32)
            rs_out = dram.tile([TPC, C], dt.float32)

            orig_c = cst.tile([P, C], dt.float32)   # original-head logits
            m_sb = cst.tile([P, H], dt.bfloat16)    # pooled m for my tokens

            # ================= Phase 1: mean pool =================
            SS = 8  # seq positions per DMA tile
            with (
                tc.tile_pool(name="hsp", bufs=3) as hsp,
                tc.tile_pool(name="ps1", bufs=1, space="PSUM") as ps1,
            ):
                m_ps0 = ps1.tile([P, 512], dt.float32)
                m_ps1 = ps1.tile([P, 512], dt.float32)
                m_halves = [m_ps0, m_ps1]
                for s0 in range(0, S, SS):
                    hs_t = hsp.tile([P, SS, H], dt.bfloat16)
                    nc.sync.dma_start(out=hs_t, in_=hsh[:, s0:s0 + SS, :])
                    for si in range(SS):
                        s = s0 + si
                        for nh in range(2):
                            nc.tensor.matmul(
                                m_halves[nh][:, :],
                                ident_b[:, :],
                                hs_t[:, si, nh * 512:(nh + 1) * 512],
                                start=(s == 0), stop=(s == S - 1),
                            )
                # m = sum/S, cast to bf16
                for nh in range(2):
                    nc.scalar.activation(
                        out=m_sb[:, nh * 512:(nh + 1) * 512],
                        in_=m_halves[nh][:, :],
                        func=mybir.ActivationFunctionType.Copy,
                        scale=1.0 / S,
                    )
                nc.sync.dma_start(out=ag_in[:, :], in_=m_sb)

            # ================= Phase 1b: original head =================
            with (
                tc.tile_pool(name="wdp", bufs=2) as wdp,
                tc.tile_pool(name="sb1", bufs=2) as sb1,
                tc.tile_pool(name="ps1b", bufs=1, space="PSUM") as ps1b,
                tc.tile_pool(name="psT1", bufs=2, space="PSUM") as psT1,
            ):
                clsT_sb = sb1.tile([P, 8, TPC], dt.float32)
                nc.sync.dma_start(
                    out=clsT_sb,
                    in_=clsT.ap().rearrange("(hc hp) t -> hp hc t", hp=P))
                bd_sb = sb1.tile([P, H], dt.float32)
                nc.sync.dma_start(out=bd_sb, in_=bd[:, :].to_broadcast((P, H)))
                og_ps0 = ps1b.tile([P, 512], dt.float32)
                og_ps1 = ps1b.tile([P, 512], dt.float32)
                og_halves = [og_ps0, og_ps1]
                for hc in range(8):
                    wd_t = wdp.tile([P, H], dt.float32)
                    nc.sync.dma_start(out=wd_t, in_=Wd[hc * P:(hc + 1) * P, :])
                    for nh in range(2):
                        nc.tensor.matmul(
                            og_halves[nh][:, :],
                            clsT_sb[:, hc, :],
                            wd_t[:, nh * 512:(nh + 1) * 512],
                            start=(hc == 0), stop=(hc == 7),
                        )
                t0 = sb1.tile([P, H], dt.float32)
                for nh in range(2):
                    nc.vector.tensor_add(
                        t0[:, nh * 512:(nh + 1) * 512],
                        og_halves[nh][:, :],
                        bd_sb[:, nh * 512:(nh + 1) * 512],
                    )
                tnh = sb1.tile([P, H], dt.float32)
                nc.scalar.activation(out=tnh, in_=t0,
                                     func=mybir.ActivationFunctionType.Tanh)
                tnhT = sb1.tile([P, 8, TPC], dt.float32)
                for hc in range(8):
                    tp_ps = psT1.tile([P, P], dt.float32)
                    nc.tensor.transpose(tp_ps[:, :],
                                        tnh[:, hc * P:(hc + 1) * P], ident_f[:, :])
                    nc.vector.tensor_copy(out=tnhT[:, hc, :], in_=tp_ps)
                Wo_sb = sb1.tile([P, 8, C], dt.float32)
                nc.sync.dma_start(
                    out=Wo_sb, in_=Wo.ap().rearrange("(hc hp) c -> hp hc c", hp=P))
                bo_sb = sb1.tile([P, C], dt.float32)
                nc.sync.dma_start(out=bo_sb, in_=bo[:, :].to_broadcast((P, C)))
                og2_ps = ps1b.tile([P, C], dt.float32)
                for hc in range(8):
                    nc.tensor.matmul(og2_ps[:, :], tnhT[:, hc, :], Wo_sb[:, hc, :],
                                     start=(hc == 0), stop=(hc == 7))
                nc.vector.tensor_add(orig_c[:, :], og2_ps[:, :], bo_sb[:, :])

            # ================= AllGather m =================
            nc.gpsimd.collective_compute(
                "AllGather", mybir.AluOpType.bypass, replica_groups=rg,
                ins=[ag_in[:, :].opt()], outs=[ag_out[:, :].opt()],
            )

            # ================= Phase 2: experts =================
            with tc.tile_pool(name="p2c", bufs=1) as p2c:
                m_full = p2c.tile([P, 8, H], dt.bfloat16)
                nc.sync.dma_start(
                    out=m_full,
                    in_=ag_out[:, :].rearrange("(tc tp) h -> tp tc h", tp=P))
                Sg_sb = p2c.tile([P, 8, SLOTS], dt.bfloat16)
                nc.sync.dma_start(
                    out=Sg_sb,
                    in_=Sg.ap().rearrange("(tc tp) j -> tp tc j", tp=P))
                SgT_sb = p2c.tile([P, JCH, B], dt.float32)
                nc.sync.dma_start(
                    out=SgT_sb,
                    in_=SgT.ap().rearrange("(jc jp) t -> jp jc t", jp=P))
                We1_sb = p2c.tile([P, EPC * 8, HID], dt.bfloat16)
                nc.sync.dma_start(
                    out=We1_sb,
                    in_=We1.ap().rearrange("e (hc hp) f -> hp (e hc) f", hp=P))
                W2p_sb = p2c.tile([P, EPC, 8, C], dt.float32)
                nc.sync.dma_start(
                    out=W2p_sb,
                    in_=W2p.ap().rearrange("e (hc hp) c -> hp e hc c", hp=P))
                be1_sb = p2c.tile([P, EPC, HID], dt.float32)
                nc.sync.dma_start(out=be1_sb,
                                  in_=be1[:, :, :].to_broadcast((P, EPC, HID)))
                g1_sb = p2c.tile([P, EPC, HID], dt.float32)
                nc.sync.dma_start(out=g1_sb,
                                  in_=g1[:, :, :].to_broadcast((P, EPC, HID)))
                beta1_sb = p2c.tile([P, EPC, HID], dt.float32)
                nc.sync.dma_start(out=beta1_sb,
                                  in_=beta1[:, :, :].to_broadcast((P, EPC, HID)))
                b2p_sb = p2c.tile([P, EPC, C], dt.float32)
                nc.sync.dma_start(out=b2p_sb,
                                  in_=b2p[:, :, :].to_broadcast((P, EPC, C)))
                wsl_sb = p2c.tile([P, JCH], dt.float32)
                nc.sync.dma_start(out=wsl_sb, in_=wsl[:, :])

                mgT_sb = p2c.tile([P, 8, SLOTS], dt.bfloat16)
                h1T_sb = p2c.tile([P, 8, SLOTS], dt.float32)
                wlog_sb = p2c.tile([P, JCH, C], dt.float32)

                # gather: mgT[h, j] = sum_t m[t, h] * Sg[t, j]
                with tc.tile_pool(name="psA", bufs=2, space="PSUM") as psA:
                    for hc in range(8):
                        for jh in range(2):
                            mgT_ps = psA.tile([P, 384], dt.float32)
                            for tcn in range(8):
                                nc.tensor.matmul(
                                    mgT_ps[:, :],
                                    m_full[:, tcn, hc * P:(hc + 1) * P],
                                    Sg_sb[:, tcn, jh * 384:(jh + 1) * 384],
                                    start=(tcn == 0), stop=(tcn == 7),
                                )
                            nc.vector.tensor_copy(
                                out=mgT_sb[:, hc, jh * 384:(jh + 1) * 384],
                                in_=mgT_ps)

                # expert MLP per slot chunk
                with (
                    tc.tile_pool(name="psB", bufs=2, space="PSUM") as psB,
                    tc.tile_pool(name="psT2", bufs=2, space="PSUM") as psT2,
                    tc.tile_pool(name="psC", bufs=2, space="PSUM") as psC,
                    tc.tile_pool(name="sbB", bufs=2) as sbB,
                ):
                    for jc in range(JCH):
                        e = jc // 3
                        h1_ps = psB.tile([P, HID], dt.float32)
                        for nh in range(2):
                            for hc in range(8):
                                nc.tensor.matmul(
                                    h1_ps[:, nh * 512:(nh + 1) * 512],
                                    mgT_sb[:, hc, jc * P:(jc + 1) * P],
                                    We1_sb[:, e * 8 + hc, nh * 512:(nh + 1) * 512],
                                    start=(hc == 0), stop=(hc == 7),
                                )
                        t_h1 = sbB.tile([P, HID], dt.float32)
                        nc.vector.tensor_add(
                            t_h1[:, :], h1_ps[:, :],
                            be1_sb[:, e, :])
                        stats = sbB.tile([P, 2, 6], dt.float32)
                        for sg in range(2):
                            nc.vector.bn_stats(
                                out=stats[:, sg, :],
                                in_=t_h1[:, sg * 512:(sg + 1) * 512])
                        mv = sbB.tile([P, 2], dt.float32)
                        nc.vector.bn_aggr(out=mv, in_=stats)
                        nc.scalar.activation(
                            out=mv[:, 1:2], in_=mv[:, 1:2],
                            func=mybir.ActivationFunctionType.Sqrt,
                            bias=eps_sb[:, :], scale=1.0)
                        nc.vector.reciprocal(out=mv[:, 1:2], in_=mv[:, 1:2])
                        nc.vector.tensor_scalar(
                            out=t_h1[:, :], in0=t_h1[:, :],
                            scalar1=mv[:, 0:1], scalar2=mv[:, 1:2],
                            op0=mybir.AluOpType.subtract,
                            op1=mybir.AluOpType.mult)
                        nc.vector.tensor_mul(
                            t_h1[:, :], t_h1[:, :],
                            g1_sb[:, e, :])
                        nc.vector.tensor_add(
                            t_h1[:, :], t_h1[:, :],
                            beta1_sb[:, e, :])
                        h1g = sbB.tile([P, HID], dt.float32)
                        nc.scalar.activation(
                            out=h1g, in_=t_h1,
                            func=mybir.ActivationFunctionType.Gelu)
                        for hc in range(8):
                            tp2 = psT2.tile([P, P], dt.float32)
                            nc.tensor.transpose(
                                tp2[:, :], h1g[:, hc * P:(hc + 1) * P],
                                ident_f[:, :])
                            nc.vector.tensor_copy(
                                out=h1T_sb[:, hc, jc * P:(jc + 1) * P], in_=tp2)
                        lg_ps = psC.tile([P, C], dt.float32)
                        for hc in range(8):
                            nc.tensor.matmul(
                                lg_ps[:, :],
                                h1T_sb[:, hc, jc * P:(jc + 1) * P],
                                W2p_sb[:, e, hc, :],
                                start=(hc == 0), stop=(hc == 7),
                            )
                        t_lg = sbB.tile([P, C], dt.float32)
                        nc.vector.tensor_add(
                            t_lg[:, :], lg_ps[:, :],
                            b2p_sb[:, e, :])
                        nc.vector.tensor_scalar_mul(
                            out=wlog_sb[:, jc, :], in0=t_lg[:, :],
                            scalar1=wsl_sb[:, jc:jc + 1])

                # scatter-add: partial[t, c] = sum_j SgT[j, t]^T wlog[j, c]
                with (
                    tc.tile_pool(name="psD", bufs=2, space="PSUM") as psD,
                    tc.tile_pool(name="sbD", bufs=2) as sbD,
                ):
                    for tcn in range(8):
                        part_ps = psD.tile([P, C], dt.float32)
                        for jc in range(JCH):
                            nc.tensor.matmul(
                                part_ps[:, :],
                                SgT_sb[:, jc, tcn * P:(tcn + 1) * P],
                                wlog_sb[:, jc, :],
                                start=(jc == 0), stop=(jc == JCH - 1),
                            )
                        part_sb = sbD.tile([P, C], dt.float32)
                        nc.vector.tensor_copy(out=part_sb, in_=part_ps)
                        nc.sync.dma_start(
                            out=rs_in[tcn * P:(tcn + 1) * P, :], in_=part_sb)

            # ================= ReduceScatter =================
            nc.gpsimd.collective_compute(
                "ReduceScatter", mybir.AluOpType.add, replica_groups=rg,
                ins=[rs_in[:, :].opt()], outs=[rs_out[:, :].opt()],
            )

            # ================= Final classifier =================
            with (
                tc.tile_pool(name="sbF", bufs=1) as sbF,
                tc.tile_pool(name="psE", bufs=1, space="PSUM") as psE,
            ):
                moe_sb = sbF.tile([P, C], dt.float32)
                nc.sync.dma_start(out=moe_sb, in_=rs_out[:, :])
                comb = sbF.tile([P, 2 * C], dt.float32)
                nc.vector.tensor_copy(out=comb[:, 0:C], in_=orig_c)
                nc.vector.tensor_copy(out=comb[:, C:2 * C], in_=moe_sb)
                cT_ps = psE.tile([2 * C, P], dt.float32)
                nc.tensor.transpose(cT_ps[:, :], comb[:, :], ident_f[:, :])
                cT = sbF.tile([2 * C, P], dt.float32)
                nc.vector.tensor_copy(out=cT, in_=cT_ps)
                Wf1_sb = sbF.tile([2 * C, C], dt.float32)
                nc.sync.dma_start(out=Wf1_sb, in_=Wf1[:, :])
                bf1_sb = sbF.tile([P, C], dt.float32)
                nc.sync.dma_start(out=bf1_sb, in_=bf1[:, :].to_broadcast((P, C)))
                gf_sb = sbF.tile([P, C], dt.float32)
                nc.sync.dma_start(out=gf_sb, in_=gf[:, :].to_broadcast((P, C)))
                betaf_sb = sbF.tile([P, C], dt.float32)
                nc.sync.dma_start(out=betaf_sb,
                                  in_=betaf[:, :].to_broadcast((P, C)))
                Wf2_sb = sbF.tile([C, C], dt.float32)
                nc.sync.dma_start(out=Wf2_sb, in_=Wf2[:, :])
                bf2_sb = sbF.tile([P, C], dt.float32)
                nc.sync.dma_start(out=bf2_sb, in_=bf2[:, :].to_broadcast((P, C)))

                z_ps = psE.tile([P, C], dt.float32)
                nc.tensor.matmul(z_ps[:, :], cT[:, :], Wf1_sb[:, :],
                                 start=True, stop=True)
                z = sbF.tile([P, C], dt.float32)
                nc.vector.tensor_add(z[:, :], z_ps[:, :], bf1_sb[:, :])
                st3 = sbF.tile([P, 6], dt.float32)
                nc.vector.bn_stats(out=st3, in_=z[:, :])
                mv3 = sbF.tile([P, 2], dt.float32)
                nc.vector.bn_aggr(out=mv3, in_=st3)
                nc.scalar.activation(
                    out=mv3[:, 1:2], in_=mv3[:, 1:2],
                    func=mybir.ActivationFunctionType.Sqrt,
                    bias=eps_sb[:, :], scale=1.0)
                nc.vector.reciprocal(out=mv3[:, 1:2], in_=mv3[:, 1:2])
                nc.vector.tensor_scalar(
                    out=z[:, :], in0=z[:, :],
                    scalar1=mv3[:, 0:1], scalar2=mv3[:, 1:2],
                    op0=mybir.AluOpType.subtract, op1=mybir.AluOpType.mult)
                nc.vector.tensor_mul(z[:, :], z[:, :], gf_sb[:, :])
                nc.vector.tensor_add(z[:, :], z[:, :], betaf_sb[:, :])
                nc.scalar.activation(out=z, in_=z,
                                     func=mybir.ActivationFunctionType.Relu)
                zT_ps = psE.tile([C, P], dt.float32)
                nc.tensor.transpose(zT_ps[:, :], z[:, :], ident_f[:, :])
                zT = sbF.tile([C, P], dt.float32)
                nc.vector.tensor_copy(out=zT, in_=zT_ps)
                o_ps = psE.tile([P, C], dt.float32)
                nc.tensor.matmul(o_ps[:, :], zT[:, :], Wf2_sb[:, :],
                                 start=True, stop=True)
                out_sb = sbF.tile([P, C], dt.float32)
                nc.vector.tensor_add(out_sb[:, :], o_ps[:, :], bf2_sb[:, :])
                nc.sync.dma_start(out=out_ext[:, :], in_=out_sb)

    nc.compile()
    return nc


def _host_prep(inputs):
    f32 = np.float32
    hs = np.asarray(inputs["hidden_states"], dtype=f32)
    cls = hs[:, 0, :]

    # routing control-plane in f64 (top-4 selection margin is ~2e-4,
    # far above f32 rounding, so this matches the reference's selection)
    r = cls.astype(np.float64) @ np.asarray(inputs["Wr"], np.float64)
    r += np.asarray(inputs["br"], np.float64)
    part = np.argpartition(-r, TOPK, axis=1)[:, :TOPK]
    vals = np.take_along_axis(r, part, axis=1)
    w = np.exp(vals - vals.max(axis=1, keepdims=True))
    w /= w.sum(axis=1, keepdims=True)
    rw = np.zeros((B, E), np.float64)
    np.put_along_axis(rw, part, w, axis=1)

    We2 = np.asarray(inputs["We2"], np.float64)
    Wp = np.asarray(inputs["Wp"], np.float64)
    W2p_all = (We2 @ Wp).astype(f32)                      # [E, HID, C]
    b2p_all = (np.asarray(inputs["be2"], np.float64) @ Wp
               + np.asarray(inputs["bp"], np.float64)).astype(f32)  # [E, C]

    Wd_b = np.asarray(inputs["Wd"], f32)
    Wo_b = np.asarray(inputs["Wo"], f32)
    We1_b = np.asarray(inputs["We1"], f32).astype(BF16)

    in_maps = []
    for c in range(NCORES):
        t0 = c * TPC
        exps = [EPC * c + i for i in range(EPC)]
        Sg = np.zeros((B, SLOTS), BF16)
        SgT = np.zeros((SLOTS, B), f32)
        wsl = np.zeros((SLOTS,), f32)
        for i, e in enumerate(exps):
            toks = np.nonzero(rw[:, e] != 0.0)[0]
            if len(toks) > CAP:
                raise RuntimeError(
                    f"expert {e} over capacity: {len(toks)} > {CAP}")
            js = i * CAP + np.arange(len(toks))
            Sg[toks, js] = 1
            SgT[js, toks] = 1
            wsl[js] = rw[toks, e].astype(f32)
        in_maps.append({
            "hsh": hs[t0:t0 + TPC].astype(BF16),
            "clsT": np.ascontiguousarray(cls[t0:t0 + TPC].T),
            "Wd": Wd_b,
            "bd": np.asarray(inputs["bd"], f32).reshape(1, H),
            "Wo": Wo_b,
            "bo": np.asarray(inputs["bo"], f32).reshape(1, C),
            "We1": We1_b[exps],
            "be1": np.asarray(inputs["be1"], f32)[exps].reshape(1, EPC, HID),
            "g1": np.asarray(inputs["g1"], f32)[exps].reshape(1, EPC, HID),
            "beta1": np.asarray(inputs["beta1"], f32)[exps].reshape(1, EPC, HID),
            "W2p": W2p_all[exps],
            "b2p": b2p_all[exps].reshape(1, EPC, C),
            "Sg": Sg,
            "SgT": SgT,
            "wsl": np.ascontiguousarray(wsl.reshape(JCH, P).T),
            "Wf1": np.asarray(inputs["Wf1"], f32),
            "bf1": np.asarray(inputs["bf1"], f32).reshape(1, C),
            "gf": np.asarray(inputs["gf"], f32).reshape(1, C),
            "betaf": np.asarray(inputs["betaf"], f32).reshape(1, C),
            "Wf2": np.asarray(inputs["Wf2"], f32),
            "bf2": np.asarray(inputs["bf2"], f32).reshape(1, C),
        })
    return in_maps


def kernel(**inputs):
    in_maps = _host_prep(inputs)
    if "nc" not in _CACHE:
        _CACHE["nc"] = _build()
    try:
        res = run_bass_kernel_spmd(_CACHE["nc"], in_maps,
                                   core_ids=list(range(NCORES)))
    except ModuleNotFoundError:
        # BASS_TRACE set but the axon NTFF hook module is absent on this
        # client — rerun untraced
        import os
        os.environ["BASS_NEVER_TRACE"] = "1"
        res = run_bass_kernel_spmd(_CACHE["nc"], in_maps,
                                   core_ids=list(range(NCORES)))
    _CACHE["last_results"] = res
    return np.concatenate([res.results[c]["out"] for c in range(NCORES)],
                          axis=0).astype(np.float32)
